# revision 9
# baseline (speedup 1.0000x reference)
"""Multi-head attention kernel for Trainium2, 8 NeuronCores.

Problem: B=2, S=2048, D=1024, H=16 heads, AD=64.
  qh = q @ W_Q ... (B,S,D)->(B,H,S,AD); attn = softmax(qh kh^T / 8)
  ctx = attn @ vh; x = merge_heads(ctx) @ W_O.  Returns (x, attn).

Sharding: batch x head-group.  Core c handles batch b=c//4 and heads
[4g, 4g+4) with g=c%4 (column block of W_Q/W_K/W_V, row block of W_O).
Partial outputs of the W_O projection are summed on the host (4 partials
per batch); attn slices are disjoint.

Per-core dataflow (S=2048, D=1024, C=256 local head cols, AD=64):
  B: stream q/k/v row-slabs, PE-transpose to get x^T (contraction dim on
     partitions), project:  qh^T, kh^T  [C, S] layouts, vh [S, C] layout.
  C: per (head, 128-row tile): logits via PE (K=64), exp on ACT with
     fused row-sum (no max subtraction: |logits| < ~10 so fp32 exp is
     safe), reciprocal + row-scale on DVE, DMA out normalized attn;
     PE-transpose the normalized attn tiles, accumulate ctx^T = vh^T P^T.
  D: x_partial = ctx @ W_O rows via PE, DMA out.
All matmuls/transposes run as float32r (full PE rate at N>=256).
"""

import numpy as np

import concourse.bacc as bacc
import concourse.mybir as mybir
import concourse.tile as tile
from concourse.bass_utils import run_bass_kernel_spmd
from concourse.masks import make_identity

B, S, D, H, AD = 2, 2048, 1024, 16, 64
NH = 4            # heads per core
C = NH * AD       # 256 local head columns
P = 128
F32 = mybir.dt.float32
F32R = mybir.dt.float32r
EXP = mybir.ActivationFunctionType.Exp
COPY = mybir.ActivationFunctionType.Copy

_CACHE: dict = {}


def _r(ap):
    return ap.bitcast(F32R)


def _emit(nc, tc, ctx, io):
    xq, xk, xv, wq, wk, wv, wo, attn, xp = io

    consts = ctx.enter_context(tc.tile_pool(name="consts", bufs=1))
    persist = ctx.enter_context(tc.tile_pool(name="persist", bufs=1))

    ident = consts.tile([P, P], F32)
    make_identity(nc, ident)
    ident_r = consts.tile([P, P], F32R)
    nc.vector.tensor_copy(ident_r, ident)

    wo_sb = consts.tile([P, 2, D], F32R)
    nc.sync.dma_start(out=wo_sb, in_=_r(wo.rearrange("(c2 p) d -> p c2 d", p=P)))

    # --- persistent per-core activations ---
    qhT = persist.tile([P, 2, S], F32R)   # [c%128, c//128, s]
    khT = persist.tile([P, 2, S], F32R)
    vh = persist.tile([P, 16, C], F32R)   # [sk%128, sk//128, c]
    ctxT = persist.tile([P, 2, S], F32R)  # like qhT layout

    # ---------------- phase B: transpose + project ----------------
    xin = {"q": xq, "k": xk, "v": xv}
    with (
        tc.tile_pool(name="wqkv", bufs=1) as w_pool,
        tc.tile_pool(name="xt", bufs=2) as xt_pool,
        tc.tile_pool(name="xT", bufs=2) as xT_pool,
        tc.tile_pool(name="psum_tr", bufs=2, space="PSUM") as psum_tr,
        tc.tile_pool(name="psum_pj", bufs=2, space="PSUM") as psum_pj,
    ):
        w_sb = {}
        for name, w in (("q", wq), ("k", wk), ("v", wv)):
            t = w_pool.tile([P, 8, C], F32R, tag=f"w{name}")
            nc.sync.dma_start(out=t, in_=_r(w.rearrange("(dc p) c -> p dc c", p=P)))
            w_sb[name] = t

        for j in range(4):          # slab of 512 rows
            for tname in ("q", "k", "v"):
                xt = xt_pool.tile([P, 4, D], F32R, tag="xt")
                nc.sync.dma_start(
                    out=xt,
                    in_=_r(xin[tname][j * 512:(j + 1) * 512, :].rearrange(
                        "(s2 p) d -> p s2 d", p=P)),
                )
                xT = xT_pool.tile([P, 8, 512], F32R, tag="xT")
                for dc in range(8):
                    pt = psum_tr.tile([P, 512], F32, tag="pt")
                    for s2 in range(4):
                        nc.tensor.transpose(
                            _r(pt[:, s2 * P:(s2 + 1) * P]),
                            xt[:, s2, dc * P:(dc + 1) * P],
                            ident_r[:, :],
                        )
                    nc.vector.tensor_copy(xT[:, dc, :], pt)
                if tname == "v":
                    # vh[sk, c] = sum_d v[sk, d] W_V[d, c]
                    for s2 in range(4):
                        pv = psum_pj.tile([P, C], F32, tag="pv")
                        for dc in range(8):
                            nc.tensor.matmul(
                                pv,
                                xT[:, dc, s2 * P:(s2 + 1) * P],
                                w_sb["v"][:, dc, :],
                                start=(dc == 0),
                                stop=(dc == 7),
                            )
                        nc.scalar.copy(vh[:, j * 4 + s2, :], pv)
                else:
                    # qh^T[c, s] = sum_d W[d, c] x^T[d, s]
                    dst = qhT if tname == "q" else khT
                    for c2 in range(2):
                        pp = psum_pj.tile([P, 512], F32, tag="pp")
                        for dc in range(8):
                            nc.tensor.matmul(
                                pp,
                                w_sb[tname][:, dc, c2 * P:(c2 + 1) * P],
                                xT[:, dc, :],
                                start=(dc == 0),
                                stop=(dc == 7),
                            )
                        nc.scalar.copy(dst[:, c2, j * 512:(j + 1) * 512], pp)

    # ---------------- phase C: attention ----------------
    SQ4 = 256  # attnT slab width (sq columns per ctx matmul)
    with (
        tc.tile_pool(name="u", bufs=2) as u_pool,
        tc.tile_pool(name="att", bufs=3) as att_pool,
        tc.tile_pool(name="attnT", bufs=2) as attnT_pool,
        tc.tile_pool(name="small", bufs=4) as small,
        tc.tile_pool(name="psum_l", bufs=2, space="PSUM") as psum_l,
        tc.tile_pool(name="psum_pt", bufs=3, space="PSUM") as psum_pt,
        tc.tile_pool(name="psum_c", bufs=1, space="PSUM") as psum_c,
    ):
        _phase_c(nc, tc, qhT, khT, vh, ctxT, attn, ident_r,
                 u_pool, att_pool, attnT_pool, small, psum_l, psum_pt, psum_c, SQ4)

    # ---------------- phase D: output projection ----------------
    x_pool = ctx.enter_context(tc.tile_pool(name="x", bufs=2))
    psum_x = ctx.enter_context(tc.tile_pool(name="psum_x", bufs=2, space="PSUM"))
    for i in range(16):
        px = psum_x.tile([P, D], F32, tag="px")
        for c2 in range(2):
            for n2 in range(2):
                nc.tensor.matmul(
                    px[:, n2 * 512:(n2 + 1) * 512],
                    ctxT[:, c2, i * P:(i + 1) * P],
                    wo_sb[:, c2, n2 * 512:(n2 + 1) * 512],
                    start=(c2 == 0),
                    stop=(c2 == 1),
                )
        xs = x_pool.tile([P, D], F32, tag="xs")
        nc.scalar.copy(xs, px)
        nc.sync.dma_start(out=xp[i * P:(i + 1) * P, :], in_=xs)


def _phase_c(nc, tc, qhT, khT, vh, ctxT, attn, ident_r,
             u_pool, att_pool, attnT_pool, small, psum_l, psum_pt, psum_c, SQ4):
    for h in range(NH):
        hp0 = (h % 2) * 64
        hc2 = h // 2
        attnT = None
        for i in range(16):  # 128-row tile of queries
            # logits + exp + rowsum, in two 1024-wide halves
            u = u_pool.tile([P, S], F32, tag="u")
            rs = small.tile([P, 2], F32, tag="rs")
            for half in range(2):
                pl = psum_l.tile([P, 1024], F32, tag="pl")
                for n2 in range(2):
                    nc.tensor.matmul(
                        pl[:, n2 * 512:(n2 + 1) * 512],
                        qhT[hp0:hp0 + 64, hc2, i * P:(i + 1) * P],
                        khT[hp0:hp0 + 64, hc2,
                            half * 1024 + n2 * 512:half * 1024 + (n2 + 1) * 512],
                        start=True,
                        stop=True,
                    )
                nc.scalar.activation(
                    u[:, half * 1024:(half + 1) * 1024], pl, EXP,
                    scale=0.125, accum_out=rs[:, half:half + 1],
                )
            rtot = small.tile([P, 1], F32, tag="rtot")
            nc.vector.tensor_add(rtot, rs[:, 0:1], rs[:, 1:2])
            rcp = small.tile([P, 1], F32, tag="rcp")
            nc.vector.reciprocal(rcp, rtot)

            att = att_pool.tile([P, S], F32R, tag="att")
            nc.vector.tensor_scalar_mul(att, u, rcp)
            nc.sync.dma_start(out=_r(attn[h, i * P:(i + 1) * P, :]), in_=att)

            # transpose normalized attn for the ctx matmul
            if i % 2 == 0:
                attnT = attnT_pool.tile([P, 16, SQ4], F32R, tag="attnT")
            assert attnT is not None
            for lq in range(4):
                pt2 = psum_pt.tile([P, 4, P], F32, tag="pt2")
                for l4 in range(4):
                    nc.tensor.transpose(
                        _r(pt2[:, l4, :]),
                        att[:, (lq * 4 + l4) * P:(lq * 4 + l4 + 1) * P],
                        ident_r[:, :],
                    )
                dst = attnT[:, lq * 4:(lq + 1) * 4, (i % 2) * P:(i % 2 + 1) * P]
                if (i + lq) % 2 == 0:
                    nc.scalar.copy(dst, pt2)
                else:
                    nc.vector.tensor_copy(dst, pt2)

            if i % 2 == 1:
                # ctx^T[c_h, sq] = sum_sk vh[sk, c_h] attn^T[sk, sq]
                pc = psum_c.tile([64, SQ4], F32, tag="pc")
                for l in range(16):
                    nc.tensor.matmul(
                        pc,
                        vh[:, l, h * 64:(h + 1) * 64],
                        attnT[:, l, :],
                        start=(l == 0),
                        stop=(l == 15),
                    )
                nc.vector.tensor_copy(
                    ctxT[hp0:hp0 + 64, hc2, (i - 1) * P:(i + 1) * P], pc)


def _build():
    if "nc" in _CACHE:
        return _CACHE["nc"]
    from contextlib import ExitStack

    nc = bacc.Bacc("TRN2", target_bir_lowering=False, debug=False)
    xq = nc.declare_dram_parameter("xq", [S, D], F32, isOutput=False).ap()
    xk = nc.declare_dram_parameter("xk", [S, D], F32, isOutput=False).ap()
    xv = nc.declare_dram_parameter("xv", [S, D], F32, isOutput=False).ap()
    wq = nc.declare_dram_parameter("wq", [D, C], F32, isOutput=False).ap()
    wk = nc.declare_dram_parameter("wk", [D, C], F32, isOutput=False).ap()
    wv = nc.declare_dram_parameter("wv", [D, C], F32, isOutput=False).ap()
    wo = nc.declare_dram_parameter("wo", [C, D], F32, isOutput=False).ap()
    attn = nc.declare_dram_parameter("attn", [NH, S, S], F32, isOutput=True).ap()
    xp = nc.declare_dram_parameter("xp", [S, D], F32, isOutput=True).ap()

    with tile.TileContext(nc) as tc, ExitStack() as ctx:
        _emit(nc, tc, ctx, (xq, xk, xv, wq, wk, wv, wo, attn, xp))
    nc.compile()
    _CACHE["nc"] = nc
    return nc


def make_in_maps(q, k, v, W_Q, W_K, W_V, W_O):
    q, k, v, W_Q, W_K, W_V, W_O = (
        np.asarray(a, dtype=np.float32) for a in (q, k, v, W_Q, W_K, W_V, W_O))
    in_maps = []
    for c in range(8):
        b, g = c // 4, c % 4
        cs = slice(g * C, (g + 1) * C)
        in_maps.append({
            "xq": np.ascontiguousarray(q[b]),
            "xk": np.ascontiguousarray(k[b]),
            "xv": np.ascontiguousarray(v[b]),
            "wq": np.ascontiguousarray(W_Q[:, cs]),
            "wk": np.ascontiguousarray(W_K[:, cs]),
            "wv": np.ascontiguousarray(W_V[:, cs]),
            "wo": np.ascontiguousarray(W_O[cs, :]),
        })
    return in_maps


def combine(results):
    attn = np.empty((B, H, S, S), np.float32)
    x = np.zeros((B, S, D), np.float32)
    for c in range(8):
        b, g = c // 4, c % 4
        attn[b, g * NH:(g + 1) * NH] = results[c]["attn"]
        x[b] += results[c]["xp"]
    return x, attn


def kernel(q, k, v, W_Q, W_K, W_V, W_O):
    nc = _build()
    in_maps = make_in_maps(q, k, v, W_Q, W_K, W_V, W_O)
    res = run_bass_kernel_spmd(nc, in_maps, core_ids=list(range(8)))
    return combine(res.results)
